# revision 22
# baseline (speedup 1.0000x reference)
"""DGCN kernel for Trainium2 (8 NeuronCores, data-parallel over batch).

Reference computation (per sample):
  h   = conv1x1(x)                                   # [C,N,T]
  hsum= h.sum(T)                                     # = W @ x.sum(T) + T*b
  a1  = softmax(relu(hsum.T @ memory * s))           # [N,N]
  a2  = softmax(relu(hsum.T @ hsum * s))             # [N,N]
  adj = softmax(fc_w0*a1 + fc_w1*a2 + fc_b)          # [N,N]
  adj = topk_mask(adj, K) * adj                      # keep K largest/row
  g1  = h  (.) adj ; g2 = g1 (.) adj                 # node contraction
  z   = gcn_w @ [g1;g2] + gcn_b
  out = z*emb + x

Structure (v3):
 - hT[n,t,c] comes straight out of per-t matmuls lhsT=x[:,:,t], rhs=WcT
   (conv fused into the transpose), bias added at eviction.
 - Adjacency pipeline is max-free (exp accumulators give the softmax
   denominators; values are small enough that exp never overflows), the
   softmax1/2 normalizers fold into per-partition combine scalars, top-k
   runs on unnormalized e3, and a single gpsimd normalize_recip applies
   mask/z3 writing bf16 adjB. All adjacency matmuls in bf16.
 - Diffusion 2 consumes g1T t-slices as lhsT against adjB so g2 lands
   c-major; only g1 needs a PE back-transpose for the projection.
 - Emission is software-pipelined with a one-sample skew:
     block k: interleave[ B(k) , C/D/E(k-1) ] ; then A(k+1)
   so the PE always has diffusion matmuls while vector/scalar run the
   adjacency chain (per-engine queues execute in emission order).

Top-k trick: softmax rows have a huge tie group at the "floor" value
(entries whose relus are all exactly 0 collapse to one float). The K-th
largest always lands inside it, so the threshold equals the floor value,
computed EXACTLY by pushing a virtual 884th zero-padded column through
the identical pipeline (zero rhs column -> s=0 -> relu=0 -> exp(0)=1).
The mask is
  (v > thr) | (v == thr & prefix_count(v == thr) <= K - count(v > thr))
which reproduces jax.lax.top_k's lowest-index-first tie breaking.
"""
import math

import ml_dtypes
import numpy as np

import concourse.bass as bass
import concourse.mybir as mybir
import concourse.tile as tile
from concourse import bacc
from concourse.bass_utils import run_bass_kernel_spmd
from concourse.masks import make_identity

B, C, N, T = 32, 128, 883, 12
K = int(N * 0.8)  # 706
NCORES = 8
SPC = B // NCORES  # samples per core
SCALE = 1.0 / math.sqrt(C)
F32 = mybir.dt.float32
F16 = mybir.dt.float16
BF16 = mybir.dt.bfloat16
AX = mybir.AxisListType
OP = mybir.AluOpType
ACTF = mybir.ActivationFunctionType

NCH = (N + 127) // 128  # 7 node chunks
CH = [(j * 128, min(128, N - j * 128)) for j in range(NCH)]  # (start, size)
# free-dim chunks for the (N+1)-wide adjacency matmuls; col N is the
# zero-padded "virtual" column that carries the tie-group threshold.
MCH = [(0, 512), (512, 372)]
NF = [(0, 512), (512, 371)]  # free chunks over N real columns
CT = C * T  # 1536


def _fch(total, step=512):
    return [(f, min(step, total - f)) for f in range(0, total, step)]


def build_nc():
    nc = bacc.Bacc(None)
    x_d = nc.dram_tensor("x", [SPC, C, N, T], F32, kind="ExternalInput")
    y_d = nc.dram_tensor("y", [SPC, C, N, T], F32, kind="ExternalOutput")
    convwTb_d = nc.dram_tensor("convwTb", [C, C], BF16, kind="ExternalInput")
    bcT4_d = nc.dram_tensor("bcT4", [128, 4 * C], BF16, kind="ExternalInput")
    convb12p_d = nc.dram_tensor("convb12p", [C, 1], F32, kind="ExternalInput")
    memory_d = nc.dram_tensor("memory", [C, N], F32, kind="ExternalInput")
    fcw0_d = nc.dram_tensor("fcw0", [C, 1], F32, kind="ExternalInput")
    fcw1_d = nc.dram_tensor("fcw1", [C, 1], F32, kind="ExternalInput")
    fcb_d = nc.dram_tensor("fcb", [C, 1], F32, kind="ExternalInput")
    gw1T_d = nc.dram_tensor("gw1T", [C, C], BF16, kind="ExternalInput")
    gw2T_d = nc.dram_tensor("gw2T", [C, C], BF16, kind="ExternalInput")
    gcnbb_d = nc.dram_tensor("gcnbb", [1, C], BF16, kind="ExternalInput")
    embx_d = nc.dram_tensor("embx", [C, 1], F32, kind="ExternalInput")
    gbe_d = nc.dram_tensor("gbe", [C, 1], F32, kind="ExternalInput")

    with tile.TileContext(nc) as tc:
        with (
            tc.tile_pool(name="const", bufs=1) as constp,
            tc.tile_pool(name="persist", bufs=1) as pers,
            tc.tile_pool(name="xin", bufs=2) as xinp,
            tc.tile_pool(name="hwin", bufs=2) as hwinp,
            tc.tile_pool(name="scr", bufs=6) as scrp,
            tc.tile_pool(name="scrh", bufs=2) as scrhp,
            tc.tile_pool(name="col", bufs=8) as colp,
            tc.tile_pool(name="gcm", bufs=2) as gcmp,
            tc.tile_pool(name="outw", bufs=2) as outwp,
            tc.tile_pool(name="mmps", bufs=5, space=bass.MemorySpace.PSUM) as mmps,
            tc.tile_pool(name="tpps", bufs=2, space=bass.MemorySpace.PSUM) as tpps,
            tc.tile_pool(name="hsps", bufs=1, space=bass.MemorySpace.PSUM) as hsps,
        ):
            # ---- constants / weights ----
            identb = constp.tile([128, 128], BF16)
            make_identity(nc, identb[:])
            zeros = constp.tile([128, N], F32)
            nc.gpsimd.memset(zeros[:], 0.0)
            zerosh = zeros[:].bitcast(F16)[:, :N]
            memp = constp.tile([C, N + 1], BF16)
            nc.gpsimd.dma_start(memp[:, :N], memory_d[:])
            nc.gpsimd.memset(memp[:, N : N + 1], 0.0)
            convwTb = constp.tile_from(convwTb_d[:])
            bcT4 = constp.tile_from(bcT4_d[:])
            convb12p = constp.tile_from(convb12p_d[:])
            fcw0 = constp.tile_from(fcw0_d[:])
            fcw1 = constp.tile_from(fcw1_d[:])
            fcb = constp.tile_from(fcb_d[:])
            gw1T = constp.tile_from(gw1T_d[:])
            gw2T = constp.tile_from(gw2T_d[:])
            gcnbb = constp.tile_from(gcnbb_d[:])
            embx = constp.tile_from(embx_d[:])
            gbe = constp.tile_from(gbe_d[:])
            bcv = bcT4[:].rearrange("p (t c) -> p t c", c=C)
            bc4r = constp.tile([1, 4 * C], BF16)
            nc.sync.dma_start(bc4r[:], bcT4_d[0:1, :])
            onesr = constp.tile([1, 128], BF16)
            nc.gpsimd.memset(onesr[:], 1.0)
            ones5 = constp.tile([1, 512], BF16)
            nc.gpsimd.memset(ones5[:], 1.0)

            states = {}

            def alloc_state(s):
                hT = pers.tile([128, NCH, T, C], BF16, tag="hT", bufs=2)
                g1T = pers.tile([128, NCH, T, C], BF16, tag="g1T", bufs=1)
                g2C = pers.tile([128, NCH, T, 128], BF16, tag="g2C", bufs=1)
                adjB = pers.tile([128, NCH, N + 1], BF16, tag="adjB", bufs=2)
                hsum = pers.tile([128, N + 1], BF16, tag="hsum", bufs=2)
                states[s] = dict(
                    hT=hT, g1T=g1T, g2C=g2C, adjB=adjB, hsum=hsum,
                    xf=x_d[s].rearrange("c n t -> c (n t)"),
                    yf=y_d[s].rearrange("c n t -> c (n t)"),
                )

            # ================= stage A: conv+transpose, xsum ===============
            def A_units(s):
                alloc_state(s)
                st = states[s]
                units = []

                def chunk(j, n0, sz):
                    st2 = st
                    xb = hwinp.tile([128, CT], BF16, tag="xb")
                    nc.gpsimd.dma_start(
                        xb[:, : sz * T], st2["xf"][:, n0 * T : (n0 + sz) * T]
                    )
                    xbv = xb[:, : sz * T].rearrange("p (n t) -> p n t", t=T)
                    # hsum chunk = sum_t Wc@x_t accumulated on the PE
                    hps = hsps.tile([128, 128], F32, tag="hs")
                    for t in range(T):
                        nc.tensor.matmul(
                            hps[:, :sz], convwTb[:], xbv[:, :, t],
                            start=(t == 0), stop=(t == T - 1),
                        )
                    real = min(sz, N - n0)
                    nc.vector.tensor_scalar(
                        st2["hsum"][:, n0 : n0 + real], hps[:, :real],
                        convb12p[:], None, op0=OP.add,
                    )
                    for tg in range(T // 4):
                        ps = mmps.tile([128, 512], F32, tag="mm")
                        pv = ps.rearrange("p (t c) -> p t c", c=C)
                        bias_mm = tg % 2 == 1
                        for tt in range(4):
                            nc.tensor.matmul(
                                pv[:sz, tt, :], xbv[:, :, tg * 4 + tt],
                                convwTb[:], start=True, stop=not bias_mm,
                                skip_group_check=bias_mm,
                            )
                        dst = st2["hT"][:sz, j, tg * 4 : tg * 4 + 4]
                        if bias_mm:
                            nc.tensor.matmul(
                                ps[:sz, :], onesr[:, :sz], bc4r[:],
                                start=False, stop=True, skip_group_check=True,
                            )
                            nc.scalar.activation(dst, pv[:sz], ACTF.Copy)
                        else:
                            nc.vector.tensor_tensor(
                                dst, pv[:sz], bcv[:sz], op=OP.add
                            )

                def vcol_unit():
                    # virtual (N+1)-th column is exactly 0
                    nc.gpsimd.memset(st["hsum"][:, N : N + 1], 0.0)

                units.append(vcol_unit)
                for j, (n0, sz) in enumerate(CH):
                    units.append(lambda j=j, n0=n0, sz=sz: chunk(j, n0, sz))
                return units

            # ================= stage B: adjacency + top-k ==================
            def B_units(s):
                st = states[s]

                def chunk(j, n0, sz):
                    hsum, adjB = st["hsum"], st["adjB"]
                    lhs = hsum[:, n0 : n0 + sz]
                    e1 = scrp.tile([128, N + 1], F32, tag="scr")
                    e2 = scrp.tile([128, N + 1], F32, tag="scr")
                    for (f0, fs), rt, rhs in (
                        (MCH[0], e1, memp), (MCH[1], e1, memp),
                        (MCH[0], e2, hsum), (MCH[1], e2, hsum),
                    ):
                        ps = mmps.tile([128, 512], F32, tag="mm")
                        nc.tensor.matmul(
                            ps[:sz, :fs], lhs, rhs[:, f0 : f0 + fs],
                            start=True, stop=True,
                        )
                        # relu(s*scale) -- matches reference op order
                        nc.scalar.activation(
                            rt[:sz, f0 : f0 + fs], ps[:sz, :fs], ACTF.Relu,
                            scale=SCALE,
                        )
                    # max-free softmax pieces: e = exp(relu), z from accum
                    z1 = colp.tile([128, 1], F32, tag="z1")
                    z2 = colp.tile([128, 1], F32, tag="z2")
                    z3 = colp.tile([128, 1], F32, tag="z3")
                    s0 = colp.tile([128, 1], F32, tag="s0")
                    s1 = colp.tile([128, 1], F32, tag="s1")
                    nc.scalar.activation(
                        e1[:sz], e1[:sz], ACTF.Exp, accum_out=z1[:sz]
                    )
                    nc.scalar.activation(
                        e2[:sz], e2[:sz], ACTF.Exp, accum_out=z2[:sz]
                    )
                    # z excludes the virtual column's exp(0)=1
                    nc.vector.tensor_sub(z1[:sz], z1[:sz], e1[:sz, N : N + 1])
                    nc.vector.tensor_sub(z2[:sz], z2[:sz], e2[:sz, N : N + 1])
                    # softmax1/2 normalization folds into combine scalars
                    nc.gpsimd.normalize_recip(s0[:sz], fcw0[:sz], z1[:sz])
                    nc.gpsimd.normalize_recip(s1[:sz], fcw1[:sz], z2[:sz])
                    t2 = scrp.tile([128, N + 1], F32, tag="scr")
                    nc.scalar.activation(
                        t2[:sz], e2[:sz], ACTF.Copy, scale=s1[:sz]
                    )
                    nc.vector.scalar_tensor_tensor(
                        e1[:sz], e1[:sz], s0[:sz], t2[:sz],
                        op0=OP.mult, op1=OP.add,
                    )
                    e3 = e1
                    nc.scalar.activation(
                        e3[:sz], e3[:sz], ACTF.Exp, bias=fcb[:sz],
                        accum_out=z3[:sz],
                    )
                    nc.vector.tensor_sub(z3[:sz], z3[:sz], e3[:sz, N : N + 1])
                    thr = e3[:sz, N : N + 1]
                    # ---- top-k mask on unnormalized e3 (scale-invariant) ----
                    gt = t2  # reuse
                    cnt = colp.tile([128, 1], F32, tag="cnt")
                    nc.vector.tensor_scalar(
                        gt[:sz, :N], e3[:sz, :N], thr, 0.0,
                        op0=OP.is_gt, op1=OP.add, accum_out=cnt[:sz],
                    )
                    eqh = scrhp.tile([128, N], F16, tag="eqh")
                    cumh = scrhp.tile([128, N], F16, tag="cumh")
                    nc.vector.tensor_scalar(
                        eqh[:sz], e3[:sz, :N], thr, None, op0=OP.is_equal
                    )
                    # cum = cnt + prefix(eq); keep ties while cum <= K (fp16
                    # stays exact: values are integers <= 883 < 2048)
                    nc.vector.tensor_tensor_scan(
                        cumh[:sz], eqh[:sz], zerosh[:sz],
                        initial=cnt[:sz], op0=OP.add, op1=OP.add,
                    )
                    nc.vector.scalar_tensor_tensor(
                        eqh[:sz], cumh[:sz], float(K), eqh[:sz],
                        op0=OP.is_le, op1=OP.mult,
                    )
                    nc.vector.tensor_add(gt[:sz, :N], gt[:sz, :N], eqh[:sz])
                    nc.vector.tensor_mul(gt[:sz, :N], e3[:sz, :N], gt[:sz, :N])
                    # adjB = masked/z3, bf16 write on gpsimd
                    nc.gpsimd.normalize_recip(
                        adjB[:sz, j, :N], gt[:sz, :N], z3[:sz]
                    )

                return [
                    (lambda j=j, n0=n0, sz=sz: chunk(j, n0, sz))
                    for j, (n0, sz) in enumerate(CH)
                ]

            # ============ stages C/D/E: diffusion + projection =============
            def CDE_units(s):
                st = states[s]
                units = []

                def c_group(kk, m0, msz, fi, f0, fs):
                    hT, g1T, adjB = st["hT"], st["g1T"], st["adjB"]
                    ps = mmps.tile([128, 512], F32, tag="mm")
                    for j, (n0, sz) in enumerate(CH):
                        rhs = hT[:sz, j].rearrange("p t c -> p (t c)")
                        nc.tensor.matmul(
                            ps[:msz, :fs], adjB[:sz, j, m0 : m0 + msz],
                            rhs[:, f0 : f0 + fs],
                            start=(j == 0), stop=(j == NCH - 1),
                        )
                    dv = g1T[:msz, kk].rearrange("p t c -> p (t c)")
                    if fi == 1:
                        nc.vector.tensor_copy(dv[:, f0 : f0 + fs], ps[:msz, :fs])
                    else:
                        nc.scalar.activation(
                            dv[:, f0 : f0 + fs], ps[:msz, :fs], ACTF.Copy
                        )

                for kk, (m0, msz) in enumerate(CH):
                    for fi, (f0, fs) in enumerate(_fch(CT)):
                        units.append(
                            lambda kk=kk, m0=m0, msz=msz, fi=fi, f0=f0, fs=fs:
                                c_group(kk, m0, msz, fi, f0, fs)
                        )

                def d_group(t, fi, f0, fs):
                    g1T, g2C, adjB = st["g1T"], st["g2C"], st["adjB"]
                    ps = mmps.tile([128, 512], F32, tag="mm")
                    for j, (n0, sz) in enumerate(CH):
                        nc.tensor.matmul(
                            ps[:, :fs], g1T[:sz, j, t, :],
                            adjB[:sz, j, f0 : f0 + fs],
                            start=(j == 0), stop=(j == NCH - 1),
                        )
                    # node-chunked g2C: contiguous 128-wide runs per chunk
                    eng = nc.scalar if (t + fi) % 2 == 0 else nc.vector
                    if fi == 0:
                        dst = g2C[:, 0:4, t, :]
                        srcv = ps[:, :512].rearrange("p (j m) -> p j m", m=128)
                        if eng is nc.scalar:
                            nc.scalar.activation(dst, srcv, ACTF.Copy)
                        else:
                            nc.vector.tensor_copy(dst, srcv)
                    else:
                        dst = g2C[:, 4:6, t, :]
                        srcv = ps[:, :256].rearrange("p (j m) -> p j m", m=128)
                        d2 = g2C[:, 6, t, : N - 512 - 256]
                        s2 = ps[:, 256 : N - 512]
                        if eng is nc.scalar:
                            nc.scalar.activation(dst, srcv, ACTF.Copy)
                            nc.scalar.activation(d2, s2, ACTF.Copy)
                        else:
                            nc.vector.tensor_copy(dst, srcv)
                            nc.vector.tensor_copy(d2, s2)

                for t in range(T):
                    for fi, (f0, fs) in enumerate(NF):
                        units.append(
                            lambda t=t, fi=fi, f0=f0, fs=fs:
                                d_group(t, fi, f0, fs)
                        )

                def e_chunk(j, n0, sz):
                    g1T, g2C = st["g1T"], st["g2C"]
                    gc1 = gcmp.tile([128, 128, T], BF16, tag="gc1")
                    for th in range(2):
                        tq = tpps.tile([128, T // 2, 128], BF16, tag="tp")
                        for tt in range(T // 2):
                            nc.tensor.transpose(
                                tq[:, tt, :sz],
                                g1T[:sz, j, th * 6 + tt, :],
                                identb[:sz, :sz],
                            )
                        src = tq[:, :, :sz].rearrange("p t m -> p m t")
                        dst = gc1[:, :sz, th * 6 : th * 6 + 6]
                        if th == 0:
                            nc.scalar.activation(dst, src, ACTF.Copy)
                        else:
                            nc.vector.tensor_copy(dst, src)
                    g1v = gc1[:, :sz].rearrange("p n t -> p (n t)")
                    g2m = g2C[:, j].rearrange("p t m -> p m t")
                    ow = outwp.tile([128, CT], F32, tag="ow")
                    x2 = xinp.tile([128, CT], F32, tag="x2")
                    nc.sync.dma_start(
                        x2[:, : sz * T], st["xf"][:, n0 * T : (n0 + sz) * T]
                    )
                    # node-aligned pieces (<=42 nodes = 504 psum cols)
                    for a0 in range(0, sz, 42):
                        an = min(42, sz - a0)
                        f0, fs = a0 * T, an * T
                        ps = mmps.tile([128, 512], F32, tag="mm")
                        nc.tensor.matmul(
                            ps[:, :fs], gcnbb[:], ones5[:, :fs],
                            start=True, stop=False, skip_group_check=True,
                        )
                        nc.tensor.matmul(
                            ps[:, :fs], gw1T[:], g1v[:, f0 : f0 + fs],
                            start=False, stop=False, skip_group_check=True,
                        )
                        nc.tensor.matmul(
                            ps[:, :fs], gw2T[:],
                            g2m[:, a0 : a0 + an, :],
                            start=False, stop=True, skip_group_check=True,
                        )
                        # ow = (z + gcn_b)*emb + x  in one pass
                        nc.vector.scalar_tensor_tensor(
                            ow[:, f0 : f0 + fs], ps[:, :fs], embx[:],
                            x2[:, f0 : f0 + fs], op0=OP.mult, op1=OP.add,
                        )
                    nc.sync.dma_start(
                        st["yf"][:, n0 * T : (n0 + sz) * T], ow[:, : sz * T]
                    )

                for j, (n0, sz) in enumerate(CH):
                    units.append(lambda j=j, n0=n0, sz=sz: e_chunk(j, n0, sz))
                return units

            # ============== software-pipelined emission ====================
            for u in A_units(0):
                u()
            for k in range(SPC + 1):
                bu = B_units(k) if k < SPC else []
                cde = CDE_units(k - 1) if k > 0 else []
                ci = 0
                step = max(1, (len(cde) + max(1, len(bu)) - 1) // max(1, len(bu)))
                for i, u in enumerate(bu):
                    u()
                    take = min(step, len(cde) - ci)
                    for _ in range(take):
                        cde[ci]()
                        ci += 1
                while ci < len(cde):
                    cde[ci]()
                    ci += 1
                if k + 1 < SPC:
                    for u in A_units(k + 1):
                        u()
                if k >= 1:
                    del states[k - 1]
    nc.compile()
    return nc


_NC = None


def _get_nc():
    global _NC
    if _NC is None:
        _NC = build_nc()
    return _NC


def make_in_maps(inputs):
    x = np.ascontiguousarray(np.asarray(inputs["x"], dtype=np.float32))
    conv_w = np.asarray(inputs["conv_w"], np.float32)
    conv_b = np.asarray(inputs["conv_b"], np.float32)
    memory = np.ascontiguousarray(np.asarray(inputs["memory"], np.float32))
    fc_w = np.asarray(inputs["fc_w"], np.float32)
    fc_b = np.asarray(inputs["fc_b"], np.float32)
    gcn_w = np.asarray(inputs["gcn_w"], np.float32)
    gcn_b = np.asarray(inputs["gcn_b"], np.float32)
    emb = np.asarray(inputs["emb"], np.float32).reshape(C)

    bcT4 = np.broadcast_to(
        conv_b[None, None, :], (128, 4, C)
    ).reshape(128, 4 * C)
    shared = {
        "convwTb": np.ascontiguousarray(conv_w.T).astype(ml_dtypes.bfloat16),
        "bcT4": np.ascontiguousarray(bcT4).astype(ml_dtypes.bfloat16),
        "convb12p": (T * conv_b).reshape(C, 1).copy(),
        "memory": memory,
        "fcw0": np.full((C, 1), fc_w[0, 0], np.float32),
        "fcw1": np.full((C, 1), fc_w[0, 1], np.float32),
        "fcb": np.full((C, 1), fc_b[0], np.float32),
        "gw1T": np.ascontiguousarray(gcn_w[:, :C].T).astype(ml_dtypes.bfloat16),
        "gw2T": np.ascontiguousarray(gcn_w[:, C:].T).astype(ml_dtypes.bfloat16),
        "gcnbb": gcn_b.reshape(1, C).astype(ml_dtypes.bfloat16),
        "embx": emb.reshape(C, 1).copy(),
        "gbe": (gcn_b * emb).reshape(C, 1).copy(),
    }
    return [
        {"x": np.ascontiguousarray(x[c * SPC : (c + 1) * SPC]), **shared}
        for c in range(NCORES)
    ]


def kernel(**inputs) -> np.ndarray:
    nc = _get_nc()
    in_maps = make_in_maps(inputs)
    res = run_bass_kernel_spmd(nc, in_maps, list(range(NCORES)))
    outs = [res.results[c]["y"] for c in range(NCORES)]
    return np.concatenate(outs, axis=0).astype(np.float32)


# revision 26
# speedup vs baseline: 1.0292x; 1.0292x over previous
"""DGCN kernel for Trainium2 (8 NeuronCores, data-parallel over batch).

Reference computation (per sample):
  h   = conv1x1(x)                                   # [C,N,T]
  hsum= h.sum(T)                                     # = W @ x.sum(T) + T*b
  a1  = softmax(relu(hsum.T @ memory * s))           # [N,N]
  a2  = softmax(relu(hsum.T @ hsum * s))             # [N,N]
  adj = softmax(fc_w0*a1 + fc_w1*a2 + fc_b)          # [N,N]
  adj = topk_mask(adj, K) * adj                      # keep K largest/row
  g1  = h  (.) adj ; g2 = g1 (.) adj                 # node contraction
  z   = gcn_w @ [g1;g2] + gcn_b
  out = z*emb + x

Structure (v3):
 - hT[n,t,c] comes straight out of per-t matmuls lhsT=x[:,:,t], rhs=WcT
   (conv fused into the transpose), bias added at eviction.
 - Adjacency pipeline is max-free (exp accumulators give the softmax
   denominators; values are small enough that exp never overflows), the
   softmax1/2 normalizers fold into per-partition combine scalars, top-k
   runs on unnormalized e3, and a single gpsimd normalize_recip applies
   mask/z3 writing bf16 adjB. All adjacency matmuls in bf16.
 - Diffusion 2 consumes g1T t-slices as lhsT against adjB so g2 lands
   c-major; only g1 needs a PE back-transpose for the projection.
 - Emission is software-pipelined with a one-sample skew:
     block k: interleave[ B(k) , C/D/E(k-1) ] ; then A(k+1)
   so the PE always has diffusion matmuls while vector/scalar run the
   adjacency chain (per-engine queues execute in emission order).

Top-k trick: softmax rows have a huge tie group at the "floor" value
(entries whose relus are all exactly 0 collapse to one float). The K-th
largest always lands inside it, so the threshold equals the floor value,
computed EXACTLY by pushing a virtual 884th zero-padded column through
the identical pipeline (zero rhs column -> s=0 -> relu=0 -> exp(0)=1).
The mask is
  (v > thr) | (v == thr & prefix_count(v == thr) <= K - count(v > thr))
which reproduces jax.lax.top_k's lowest-index-first tie breaking.
"""
import math

import ml_dtypes
import numpy as np

import concourse.bass as bass
import concourse.mybir as mybir
import concourse.tile as tile
from concourse import bacc
from concourse.bass_utils import run_bass_kernel_spmd
from concourse.masks import make_identity

B, C, N, T = 32, 128, 883, 12
K = int(N * 0.8)  # 706
NCORES = 8
SPC = B // NCORES  # samples per core
SCALE = 1.0 / math.sqrt(C)
F32 = mybir.dt.float32
F16 = mybir.dt.float16
BF16 = mybir.dt.bfloat16
AX = mybir.AxisListType
OP = mybir.AluOpType
ACTF = mybir.ActivationFunctionType

NCH = (N + 127) // 128  # 7 node chunks
CH = [(j * 128, min(128, N - j * 128)) for j in range(NCH)]  # (start, size)
# free-dim chunks for the (N+1)-wide adjacency matmuls; col N is the
# zero-padded "virtual" column that carries the tie-group threshold.
MCH = [(0, 512), (512, 372)]
NF = [(0, 512), (512, 371)]  # free chunks over N real columns
CT = C * T  # 1536


def _fch(total, step=512):
    return [(f, min(step, total - f)) for f in range(0, total, step)]


def build_nc():
    nc = bacc.Bacc(None)
    x_d = nc.dram_tensor("x", [SPC, C, N, T], F32, kind="ExternalInput")
    y_d = nc.dram_tensor("y", [SPC, C, N, T], F32, kind="ExternalOutput")
    convwTb_d = nc.dram_tensor("convwTb", [C, C], BF16, kind="ExternalInput")
    bcT4_d = nc.dram_tensor("bcT4", [128, 4 * C], BF16, kind="ExternalInput")
    convb12p_d = nc.dram_tensor("convb12p", [C, 1], F32, kind="ExternalInput")
    memory_d = nc.dram_tensor("memory", [C, N], F32, kind="ExternalInput")
    fcw0_d = nc.dram_tensor("fcw0", [C, 1], F32, kind="ExternalInput")
    fcw1_d = nc.dram_tensor("fcw1", [C, 1], F32, kind="ExternalInput")
    fcb_d = nc.dram_tensor("fcb", [C, 1], F32, kind="ExternalInput")
    gw1T_d = nc.dram_tensor("gw1T", [C, C], BF16, kind="ExternalInput")
    gw2T_d = nc.dram_tensor("gw2T", [C, C], BF16, kind="ExternalInput")
    gcnbb_d = nc.dram_tensor("gcnbb", [1, C], BF16, kind="ExternalInput")
    embx_d = nc.dram_tensor("embx", [C, 1], F32, kind="ExternalInput")
    gbe_d = nc.dram_tensor("gbe", [C, 1], F32, kind="ExternalInput")

    with tile.TileContext(nc) as tc:
        with (
            tc.tile_pool(name="const", bufs=1) as constp,
            tc.tile_pool(name="persist", bufs=1) as pers,
            tc.tile_pool(name="xin", bufs=2) as xinp,
            tc.tile_pool(name="hwin", bufs=2) as hwinp,
            tc.tile_pool(name="scr", bufs=6) as scrp,
            tc.tile_pool(name="scrh", bufs=2) as scrhp,
            tc.tile_pool(name="col", bufs=8) as colp,
            tc.tile_pool(name="gcm", bufs=2) as gcmp,
            tc.tile_pool(name="outw", bufs=2) as outwp,
            tc.tile_pool(name="mmps", bufs=6, space=bass.MemorySpace.PSUM) as mmps,
            tc.tile_pool(name="tpps", bufs=2, space=bass.MemorySpace.PSUM) as tpps,
        ):
            # ---- constants / weights ----
            identb = constp.tile([128, 128], BF16)
            make_identity(nc, identb[:])
            zeros = constp.tile([128, N], F32)
            nc.gpsimd.memset(zeros[:], 0.0)
            zerosh = zeros[:].bitcast(F16)[:, :N]
            memp = constp.tile([C, N + 1], BF16)
            nc.gpsimd.dma_start(memp[:, :N], memory_d[:])
            nc.gpsimd.memset(memp[:, N : N + 1], 0.0)
            convwTb = constp.tile_from(convwTb_d[:])
            bcT4 = constp.tile_from(bcT4_d[:])
            convb12p = constp.tile_from(convb12p_d[:])
            fcw0 = constp.tile_from(fcw0_d[:])
            fcw1 = constp.tile_from(fcw1_d[:])
            fcb = constp.tile_from(fcb_d[:])
            gw1T = constp.tile_from(gw1T_d[:])
            gw2T = constp.tile_from(gw2T_d[:])
            gcnbb = constp.tile_from(gcnbb_d[:])
            embx = constp.tile_from(embx_d[:])
            gbe = constp.tile_from(gbe_d[:])
            bcv = bcT4[:].rearrange("p (t c) -> p t c", c=C)
            bc4r = constp.tile([1, 4 * C], BF16)
            nc.sync.dma_start(bc4r[:], bcT4_d[0:1, :])
            onesr = constp.tile([1, 128], BF16)
            nc.gpsimd.memset(onesr[:], 1.0)
            ones5 = constp.tile([1, 512], BF16)
            nc.gpsimd.memset(ones5[:], 1.0)

            states = {}

            def alloc_state(s):
                hT = pers.tile([128, NCH, T, C], BF16, tag="hT", bufs=2)
                g1T = pers.tile([128, NCH, T, C], BF16, tag="g1T", bufs=1)
                g2C = pers.tile([128, NCH, T, 128], BF16, tag="g2C", bufs=1)
                adjB = pers.tile([128, NCH, N + 1], BF16, tag="adjB", bufs=2)
                xsum = pers.tile([128, N], F32, tag="xsum", bufs=2)
                xsumb = pers.tile([128, N + 1], BF16, tag="xsumb", bufs=2)
                hsum = pers.tile([128, N + 1], BF16, tag="hsum", bufs=2)
                states[s] = dict(
                    hT=hT, g1T=g1T, g2C=g2C, adjB=adjB, hsum=hsum,
                    xsum=xsum, xsumb=xsumb,
                    xf=x_d[s].rearrange("c n t -> c (n t)"),
                    yf=y_d[s].rearrange("c n t -> c (n t)"),
                )

            # ================= stage A: conv+transpose, xsum ===============
            def A_units(s):
                alloc_state(s)
                st = states[s]
                units = []

                def chunk(j, n0, sz):
                    st2 = st
                    xb = hwinp.tile([128, CT], BF16, tag="xb")
                    nc.gpsimd.dma_start(
                        xb[:, : sz * T], st2["xf"][:, n0 * T : (n0 + sz) * T]
                    )
                    xbv = xb[:, : sz * T].rearrange("p (n t) -> p n t", t=T)
                    nc.vector.tensor_reduce(
                        st2["xsum"][:, n0 : n0 + sz], xbv, axis=AX.X, op=OP.add
                    )
                    for tg in range(T // 4):
                        ps = mmps.tile([128, 512], F32, tag="mm")
                        pv = ps.rearrange("p (t c) -> p t c", c=C)
                        bias_mm = tg % 2 == 1
                        for tt in range(4):
                            nc.tensor.matmul(
                                pv[:sz, tt, :], xbv[:, :, tg * 4 + tt],
                                convwTb[:], start=True, stop=not bias_mm,
                                skip_group_check=bias_mm,
                            )
                        dst = st2["hT"][:sz, j, tg * 4 : tg * 4 + 4]
                        if bias_mm:
                            nc.tensor.matmul(
                                ps[:sz, :], onesr[:, :sz], bc4r[:],
                                start=False, stop=True, skip_group_check=True,
                            )
                            nc.scalar.activation(dst, pv[:sz], ACTF.Copy)
                        else:
                            nc.vector.tensor_tensor(
                                dst, pv[:sz], bcv[:sz], op=OP.add
                            )

                for j, (n0, sz) in enumerate(CH):
                    units.append(lambda j=j, n0=n0, sz=sz: chunk(j, n0, sz))

                def hsum_unit():
                    xsum, xsumb, hsum = st["xsum"], st["xsumb"], st["hsum"]
                    nc.gpsimd.memset(xsumb[:, N : N + 1], 0.0)
                    nc.scalar.activation(xsumb[:, :N], xsum[:], ACTF.Copy)
                    for f0, fs in MCH:
                        ps = mmps.tile([128, 512], F32, tag="mm")
                        nc.tensor.matmul(
                            ps[:, :fs], convwTb[:], xsumb[:, f0 : f0 + fs],
                            start=True, stop=True,
                        )
                        real = min(fs, N - f0)  # no bias on the virtual col
                        nc.vector.tensor_scalar(
                            hsum[:, f0 : f0 + real], ps[:, :real],
                            convb12p[:], None, op0=OP.add,
                        )
                        if real < fs:
                            nc.vector.tensor_copy(
                                hsum[:, f0 + real : f0 + fs], ps[:, real:fs]
                            )

                units.append(hsum_unit)
                return units

            # ================= stage B: adjacency + top-k ==================
            def B_units(s):
                st = states[s]

                def chunk(j, n0, sz):
                    hsum, adjB = st["hsum"], st["adjB"]
                    lhs = hsum[:, n0 : n0 + sz]
                    e1 = scrp.tile([128, N + 1], F32, tag="scr")
                    e2 = scrp.tile([128, N + 1], F32, tag="scr")
                    for (f0, fs), rt, rhs in (
                        (MCH[0], e1, memp), (MCH[1], e1, memp),
                        (MCH[0], e2, hsum), (MCH[1], e2, hsum),
                    ):
                        ps = mmps.tile([128, 512], F32, tag="mm")
                        nc.tensor.matmul(
                            ps[:sz, :fs], lhs, rhs[:, f0 : f0 + fs],
                            start=True, stop=True,
                        )
                        # relu(s*scale) -- matches reference op order
                        nc.scalar.activation(
                            rt[:sz, f0 : f0 + fs], ps[:sz, :fs], ACTF.Relu,
                            scale=SCALE,
                        )
                    # max-free softmax pieces: e = exp(relu), z from accum
                    z1 = colp.tile([128, 1], F32, tag="z1")
                    z2 = colp.tile([128, 1], F32, tag="z2")
                    z3 = colp.tile([128, 1], F32, tag="z3")
                    s0 = colp.tile([128, 1], F32, tag="s0")
                    s1 = colp.tile([128, 1], F32, tag="s1")
                    nc.scalar.activation(
                        e1[:sz], e1[:sz], ACTF.Exp, accum_out=z1[:sz]
                    )
                    nc.scalar.activation(
                        e2[:sz], e2[:sz], ACTF.Exp, accum_out=z2[:sz]
                    )
                    # z excludes the virtual column's exp(0)=1
                    nc.vector.tensor_sub(z1[:sz], z1[:sz], e1[:sz, N : N + 1])
                    nc.vector.tensor_sub(z2[:sz], z2[:sz], e2[:sz, N : N + 1])
                    # softmax1/2 normalization folds into combine scalars
                    nc.gpsimd.normalize_recip(s0[:sz], fcw0[:sz], z1[:sz])
                    nc.gpsimd.normalize_recip(s1[:sz], fcw1[:sz], z2[:sz])
                    t2 = scrp.tile([128, N + 1], F32, tag="scr")
                    nc.scalar.activation(
                        t2[:sz], e2[:sz], ACTF.Copy, scale=s1[:sz]
                    )
                    nc.vector.scalar_tensor_tensor(
                        e1[:sz], e1[:sz], s0[:sz], t2[:sz],
                        op0=OP.mult, op1=OP.add,
                    )
                    e3 = e1
                    nc.scalar.activation(
                        e3[:sz], e3[:sz], ACTF.Exp, bias=fcb[:sz],
                        accum_out=z3[:sz],
                    )
                    nc.vector.tensor_sub(z3[:sz], z3[:sz], e3[:sz, N : N + 1])
                    thr = e3[:sz, N : N + 1]
                    # ---- top-k mask on unnormalized e3 (scale-invariant) ----
                    gt = t2  # reuse
                    cnt = colp.tile([128, 1], F32, tag="cnt")
                    nc.vector.tensor_scalar(
                        gt[:sz, :N], e3[:sz, :N], thr, 0.0,
                        op0=OP.is_gt, op1=OP.add, accum_out=cnt[:sz],
                    )
                    eqh = scrhp.tile([128, N], F16, tag="eqh")
                    cumh = scrhp.tile([128, N], F16, tag="cumh")
                    nc.vector.tensor_scalar(
                        eqh[:sz], e3[:sz, :N], thr, None, op0=OP.is_equal
                    )
                    # cum = cnt + prefix(eq); keep ties while cum <= K (fp16
                    # stays exact: values are integers <= 883 < 2048)
                    nc.vector.tensor_tensor_scan(
                        cumh[:sz], eqh[:sz], zerosh[:sz],
                        initial=cnt[:sz], op0=OP.add, op1=OP.add,
                    )
                    nc.vector.scalar_tensor_tensor(
                        eqh[:sz], cumh[:sz], float(K), eqh[:sz],
                        op0=OP.is_le, op1=OP.mult,
                    )
                    nc.vector.tensor_add(gt[:sz, :N], gt[:sz, :N], eqh[:sz])
                    nc.vector.tensor_mul(gt[:sz, :N], e3[:sz, :N], gt[:sz, :N])
                    # adjB = masked/z3, bf16 write on gpsimd
                    nc.gpsimd.normalize_recip(
                        adjB[:sz, j, :N], gt[:sz, :N], z3[:sz]
                    )

                return [
                    (lambda j=j, n0=n0, sz=sz: chunk(j, n0, sz))
                    for j, (n0, sz) in enumerate(CH)
                ]

            # ============ stages C/D/E: diffusion + projection =============
            def CDE_units(s):
                st = states[s]
                units = []

                def c_group(kk, m0, msz, fi, f0, fs):
                    hT, g1T, adjB = st["hT"], st["g1T"], st["adjB"]
                    ps = mmps.tile([128, 512], F32, tag="mm")
                    for j, (n0, sz) in enumerate(CH):
                        rhs = hT[:sz, j].rearrange("p t c -> p (t c)")
                        nc.tensor.matmul(
                            ps[:msz, :fs], adjB[:sz, j, m0 : m0 + msz],
                            rhs[:, f0 : f0 + fs],
                            start=(j == 0), stop=(j == NCH - 1),
                        )
                    dv = g1T[:msz, kk].rearrange("p t c -> p (t c)")
                    if fi == 1:
                        nc.vector.tensor_copy(dv[:, f0 : f0 + fs], ps[:msz, :fs])
                    else:
                        nc.scalar.activation(
                            dv[:, f0 : f0 + fs], ps[:msz, :fs], ACTF.Copy
                        )

                for kk, (m0, msz) in enumerate(CH):
                    for fi, (f0, fs) in enumerate(_fch(CT)):
                        units.append(
                            lambda kk=kk, m0=m0, msz=msz, fi=fi, f0=f0, fs=fs:
                                c_group(kk, m0, msz, fi, f0, fs)
                        )

                def d_group(t, fi, f0, fs):
                    g1T, g2C, adjB = st["g1T"], st["g2C"], st["adjB"]
                    ps = mmps.tile([128, 512], F32, tag="mm")
                    for j, (n0, sz) in enumerate(CH):
                        nc.tensor.matmul(
                            ps[:, :fs], g1T[:sz, j, t, :],
                            adjB[:sz, j, f0 : f0 + fs],
                            start=(j == 0), stop=(j == NCH - 1),
                        )
                    # node-chunked g2C: contiguous 128-wide runs per chunk
                    eng = nc.scalar if (t + fi) % 2 == 0 else nc.vector
                    if fi == 0:
                        dst = g2C[:, 0:4, t, :]
                        srcv = ps[:, :512].rearrange("p (j m) -> p j m", m=128)
                        if eng is nc.scalar:
                            nc.scalar.activation(dst, srcv, ACTF.Copy)
                        else:
                            nc.vector.tensor_copy(dst, srcv)
                    else:
                        dst = g2C[:, 4:6, t, :]
                        srcv = ps[:, :256].rearrange("p (j m) -> p j m", m=128)
                        d2 = g2C[:, 6, t, : N - 512 - 256]
                        s2 = ps[:, 256 : N - 512]
                        if eng is nc.scalar:
                            nc.scalar.activation(dst, srcv, ACTF.Copy)
                            nc.scalar.activation(d2, s2, ACTF.Copy)
                        else:
                            nc.vector.tensor_copy(dst, srcv)
                            nc.vector.tensor_copy(d2, s2)

                for t in range(T):
                    for fi, (f0, fs) in enumerate(NF):
                        units.append(
                            lambda t=t, fi=fi, f0=f0, fs=fs:
                                d_group(t, fi, f0, fs)
                        )

                def e_chunk(j, n0, sz):
                    g1T, g2C = st["g1T"], st["g2C"]
                    gc1 = gcmp.tile([128, 128, T], BF16, tag="gc1")
                    for th in range(2):
                        tq = tpps.tile([128, T // 2, 128], BF16, tag="tp")
                        for tt in range(T // 2):
                            nc.tensor.transpose(
                                tq[:, tt, :sz],
                                g1T[:sz, j, th * 6 + tt, :],
                                identb[:sz, :sz],
                            )
                        src = tq[:, :, :sz].rearrange("p t m -> p m t")
                        dst = gc1[:, :sz, th * 6 : th * 6 + 6]
                        if th == 0:
                            nc.scalar.activation(dst, src, ACTF.Copy)
                        else:
                            nc.vector.tensor_copy(dst, src)
                    g1v = gc1[:, :sz].rearrange("p n t -> p (n t)")
                    g2m = g2C[:, j].rearrange("p t m -> p m t")
                    ow = outwp.tile([128, CT], F32, tag="ow")
                    x2 = xinp.tile([128, CT], F32, tag="x2")
                    nc.sync.dma_start(
                        x2[:, : sz * T], st["xf"][:, n0 * T : (n0 + sz) * T]
                    )
                    # node-aligned pieces (<=42 nodes = 504 psum cols)
                    for a0 in range(0, sz, 42):
                        an = min(42, sz - a0)
                        f0, fs = a0 * T, an * T
                        ps = mmps.tile([128, 512], F32, tag="mm")
                        nc.tensor.matmul(
                            ps[:, :fs], gcnbb[:], ones5[:, :fs],
                            start=True, stop=False, skip_group_check=True,
                        )
                        nc.tensor.matmul(
                            ps[:, :fs], gw1T[:], g1v[:, f0 : f0 + fs],
                            start=False, stop=False, skip_group_check=True,
                        )
                        nc.tensor.matmul(
                            ps[:, :fs], gw2T[:],
                            g2m[:, a0 : a0 + an, :],
                            start=False, stop=True, skip_group_check=True,
                        )
                        # ow = (z + gcn_b)*emb + x  in one pass
                        nc.vector.scalar_tensor_tensor(
                            ow[:, f0 : f0 + fs], ps[:, :fs], embx[:],
                            x2[:, f0 : f0 + fs], op0=OP.mult, op1=OP.add,
                        )
                    nc.sync.dma_start(
                        st["yf"][:, n0 * T : (n0 + sz) * T], ow[:, : sz * T]
                    )

                for j, (n0, sz) in enumerate(CH):
                    units.append(lambda j=j, n0=n0, sz=sz: e_chunk(j, n0, sz))
                return units

            # ============== software-pipelined emission ====================
            for u in A_units(0):
                u()
            for k in range(SPC + 1):
                bu = B_units(k) if k < SPC else []
                cde = CDE_units(k - 1) if k > 0 else []
                ci = 0
                step = max(1, (len(cde) + max(1, len(bu)) - 1) // max(1, len(bu)))
                for i, u in enumerate(bu):
                    u()
                    take = min(step, len(cde) - ci)
                    for _ in range(take):
                        cde[ci]()
                        ci += 1
                while ci < len(cde):
                    cde[ci]()
                    ci += 1
                if k + 1 < SPC:
                    for u in A_units(k + 1):
                        u()
                if k >= 1:
                    del states[k - 1]
    nc.compile()
    return nc


_NC = None


def _get_nc():
    global _NC
    if _NC is None:
        _NC = build_nc()
    return _NC


def make_in_maps(inputs):
    x = np.ascontiguousarray(np.asarray(inputs["x"], dtype=np.float32))
    conv_w = np.asarray(inputs["conv_w"], np.float32)
    conv_b = np.asarray(inputs["conv_b"], np.float32)
    memory = np.ascontiguousarray(np.asarray(inputs["memory"], np.float32))
    fc_w = np.asarray(inputs["fc_w"], np.float32)
    fc_b = np.asarray(inputs["fc_b"], np.float32)
    gcn_w = np.asarray(inputs["gcn_w"], np.float32)
    gcn_b = np.asarray(inputs["gcn_b"], np.float32)
    emb = np.asarray(inputs["emb"], np.float32).reshape(C)

    bcT4 = np.broadcast_to(
        conv_b[None, None, :], (128, 4, C)
    ).reshape(128, 4 * C)
    shared = {
        "convwTb": np.ascontiguousarray(conv_w.T).astype(ml_dtypes.bfloat16),
        "bcT4": np.ascontiguousarray(bcT4).astype(ml_dtypes.bfloat16),
        "convb12p": (T * conv_b).reshape(C, 1).copy(),
        "memory": memory,
        "fcw0": np.full((C, 1), fc_w[0, 0], np.float32),
        "fcw1": np.full((C, 1), fc_w[0, 1], np.float32),
        "fcb": np.full((C, 1), fc_b[0], np.float32),
        "gw1T": np.ascontiguousarray(gcn_w[:, :C].T).astype(ml_dtypes.bfloat16),
        "gw2T": np.ascontiguousarray(gcn_w[:, C:].T).astype(ml_dtypes.bfloat16),
        "gcnbb": gcn_b.reshape(1, C).astype(ml_dtypes.bfloat16),
        "embx": emb.reshape(C, 1).copy(),
        "gbe": (gcn_b * emb).reshape(C, 1).copy(),
    }
    return [
        {"x": np.ascontiguousarray(x[c * SPC : (c + 1) * SPC]), **shared}
        for c in range(NCORES)
    ]


def kernel(**inputs) -> np.ndarray:
    nc = _get_nc()
    in_maps = make_in_maps(inputs)
    res = run_bass_kernel_spmd(nc, in_maps, list(range(NCORES)))
    outs = [res.results[c]["y"] for c in range(NCORES)]
    return np.concatenate(outs, axis=0).astype(np.float32)


# revision 38
# speedup vs baseline: 1.0961x; 1.0650x over previous
"""DGCN kernel for Trainium2 (8 NeuronCores, data-parallel over batch).

Reference computation (per sample):
  h   = conv1x1(x)                                   # [C,N,T]
  hsum= h.sum(T)                                     # = W @ x.sum(T) + T*b
  a1  = softmax(relu(hsum.T @ memory * s))           # [N,N]
  a2  = softmax(relu(hsum.T @ hsum * s))             # [N,N]
  adj = softmax(fc_w0*a1 + fc_w1*a2 + fc_b)          # [N,N]
  adj = topk_mask(adj, K) * adj                      # keep K largest/row
  g1  = h  (.) adj ; g2 = g1 (.) adj                 # node contraction
  z   = gcn_w @ [g1;g2] + gcn_b
  out = z*emb + x

Structure (v3):
 - hT[n,t,c] comes straight out of per-t matmuls lhsT=x[:,:,t], rhs=WcT
   (conv fused into the transpose), bias added at eviction.
 - Adjacency pipeline is max-free (exp accumulators give the softmax
   denominators; values are small enough that exp never overflows), the
   softmax1/2 normalizers fold into per-partition combine scalars, top-k
   runs on unnormalized e3, and a single gpsimd normalize_recip applies
   mask/z3 writing bf16 adjB. All adjacency matmuls in bf16.
 - Diffusion 2 consumes g1T t-slices as lhsT against adjB so g2 lands
   c-major; only g1 needs a PE back-transpose for the projection.
 - Emission is software-pipelined with a one-sample skew:
     block k: interleave[ B(k) , C/D/E(k-1) ] ; then A(k+1)
   so the PE always has diffusion matmuls while vector/scalar run the
   adjacency chain (per-engine queues execute in emission order).

Top-k trick: softmax rows have a huge tie group at the "floor" value
(entries whose relus are all exactly 0 collapse to one float). The K-th
largest always lands inside it, so the threshold equals the floor value,
computed EXACTLY by pushing a virtual 884th zero-padded column through
the identical pipeline (zero rhs column -> s=0 -> relu=0 -> exp(0)=1).
The mask is
  (v > thr) | (v == thr & prefix_count(v == thr) <= K - count(v > thr))
which reproduces jax.lax.top_k's lowest-index-first tie breaking.
"""
import math

import ml_dtypes
import numpy as np

import concourse.bass as bass
import concourse.mybir as mybir
import concourse.tile as tile
from concourse import bacc
from concourse.bass_utils import run_bass_kernel_spmd
from concourse.masks import make_identity

B, C, N, T = 32, 128, 883, 12
K = int(N * 0.8)  # 706
NCORES = 8
SPC = B // NCORES  # samples per core
SCALE = 1.0 / math.sqrt(C)
F32 = mybir.dt.float32
F16 = mybir.dt.float16
BF16 = mybir.dt.bfloat16
FP8 = mybir.dt.float8e4
DR = mybir.MatmulPerfMode.DoubleRow
ASCL = 64.0  # adjB pre-scale: lifts the adj floor out of fp8 subnormals
AX = mybir.AxisListType
OP = mybir.AluOpType
ACTF = mybir.ActivationFunctionType

NCH = (N + 127) // 128  # 7 node chunks
CH = [(j * 128, min(128, N - j * 128)) for j in range(NCH)]  # (start, size)
# free-dim chunks for the (N+1)-wide adjacency matmuls; col N is the
# zero-padded "virtual" column that carries the tie-group threshold.
MCH = [(0, 512), (512, 372)]
NF = [(0, 512), (512, 371)]  # free chunks over N real columns
CT = C * T  # 1536


def _fch(total, step=512):
    return [(f, min(step, total - f)) for f in range(0, total, step)]


def build_nc():
    nc = bacc.Bacc(None)
    x_d = nc.dram_tensor("x", [SPC, C, N, T], F32, kind="ExternalInput")
    y_d = nc.dram_tensor("y", [SPC, C, N, T], F32, kind="ExternalOutput")
    convwTb_d = nc.dram_tensor("convwTb", [C, C], BF16, kind="ExternalInput")
    bcT4_d = nc.dram_tensor("bcT4", [128, 4 * C], BF16, kind="ExternalInput")
    convb12p_d = nc.dram_tensor("convb12p", [C, 1], F32, kind="ExternalInput")
    memory_d = nc.dram_tensor("memory", [C, N], F32, kind="ExternalInput")
    fcw0_d = nc.dram_tensor("fcw0", [C, 1], F32, kind="ExternalInput")
    fcw1_d = nc.dram_tensor("fcw1", [C, 1], F32, kind="ExternalInput")
    fcb_d = nc.dram_tensor("fcb", [C, 1], F32, kind="ExternalInput")
    gw1T_d = nc.dram_tensor("gw1T", [C, C], BF16, kind="ExternalInput")
    gw2T_d = nc.dram_tensor("gw2T", [C, C], BF16, kind="ExternalInput")
    embx_d = nc.dram_tensor("embx", [C, 1], F32, kind="ExternalInput")
    gbe_d = nc.dram_tensor("gbe", [C, 1], F32, kind="ExternalInput")

    with tile.TileContext(nc) as tc:
        with (
            tc.tile_pool(name="const", bufs=1) as constp,
            tc.tile_pool(name="persist", bufs=1) as pers,
            tc.tile_pool(name="xin", bufs=2) as xinp,
            tc.tile_pool(name="hwin", bufs=2) as hwinp,
            tc.tile_pool(name="scr", bufs=6) as scrp,
            tc.tile_pool(name="scrh", bufs=2) as scrhp,
            tc.tile_pool(name="col", bufs=8) as colp,
            tc.tile_pool(name="gcm", bufs=2) as gcmp,
            tc.tile_pool(name="outw", bufs=2) as outwp,
            tc.tile_pool(name="mmps", bufs=6, space=bass.MemorySpace.PSUM) as mmps,
            tc.tile_pool(name="tpps", bufs=2, space=bass.MemorySpace.PSUM) as tpps,
        ):
            # ---- constants / weights ----
            identb = constp.tile([128, 128], BF16)
            make_identity(nc, identb[:])
            zeros = constp.tile([128, N], F32)
            nc.gpsimd.memset(zeros[:], 0.0)
            zerosh = zeros[:].bitcast(F16)[:, :N]
            memp = constp.tile([C, N + 1], BF16)
            nc.gpsimd.dma_start(memp[:, :N], memory_d[:])
            nc.gpsimd.memset(memp[:, N : N + 1], 0.0)
            convwTb = constp.tile_from(convwTb_d[:])
            bcT4 = constp.tile_from(bcT4_d[:])
            convb12p = constp.tile_from(convb12p_d[:])
            fcw0 = constp.tile_from(fcw0_d[:])
            fcw1 = constp.tile_from(fcw1_d[:])
            fcb = constp.tile_from(fcb_d[:])
            gw1T = constp.tile_from(gw1T_d[:])
            gw2T = constp.tile_from(gw2T_d[:])
            embx = constp.tile_from(embx_d[:])
            gbe = constp.tile_from(gbe_d[:])
            bcv = bcT4[:].rearrange("p (t c) -> p t c", c=C)
            bc4r = constp.tile([1, 4 * C], BF16)
            nc.sync.dma_start(bc4r[:], bcT4_d[0:1, :])
            onesr = constp.tile([1, 128], BF16)
            nc.gpsimd.memset(onesr[:], 1.0)
            identf8 = constp.tile([128, 128], FP8)
            nc.vector.tensor_copy(identf8[:], identb[:])
            c64 = constp.tile([C, 1], F32)
            nc.gpsimd.memset(c64[:], ASCL)

            states = {}

            def alloc_state(s):
                hT = pers.tile([128, NCH, T, C], FP8, tag="hT", bufs=2)
                g1T = pers.tile([128, NCH, T, C], FP8, tag="g1T", bufs=2)
                g2C = pers.tile([128, NCH, T, 128], BF16, tag="g2C", bufs=1)
                # last dim padded to 896 so the chunk-pair (DoubleRow k-tile)
                # stride is 16-byte aligned
                adjB = pers.tile([128, NCH, 896], FP8, tag="adjB", bufs=2)
                xsum = pers.tile([128, N], F32, tag="xsum", bufs=2)
                xsumb = pers.tile([128, N + 1], BF16, tag="xsumb", bufs=2)
                hsum = pers.tile([128, N + 1], BF16, tag="hsum", bufs=2)
                states[s] = dict(
                    hT=hT, g1T=g1T, g2C=g2C, adjB=adjB, hsum=hsum,
                    xsum=xsum, xsumb=xsumb,
                    xf=x_d[s].rearrange("c n t -> c (n t)"),
                    yf=y_d[s].rearrange("c n t -> c (n t)"),
                )

            # ================= stage A: conv+transpose, xsum ===============
            def A_units(s):
                alloc_state(s)
                st = states[s]
                units = []

                def chunk(j, n0, sz):
                    st2 = st
                    xb = hwinp.tile([128, CT], BF16, tag="xb")
                    nc.gpsimd.dma_start(
                        xb[:, : sz * T], st2["xf"][:, n0 * T : (n0 + sz) * T]
                    )
                    xbv = xb[:, : sz * T].rearrange("p (n t) -> p n t", t=T)
                    nc.vector.tensor_reduce(
                        st2["xsum"][:, n0 : n0 + sz], xbv, axis=AX.X, op=OP.add
                    )
                    for tg in range(T // 4):
                        ps = mmps.tile([128, 512], F32, tag="mm")
                        pv = ps.rearrange("p (t c) -> p t c", c=C)
                        bias_mm = tg % 2 == 1
                        for tt in range(4):
                            nc.tensor.matmul(
                                pv[:sz, tt, :], xbv[:, :, tg * 4 + tt],
                                convwTb[:], start=True, stop=not bias_mm,
                                skip_group_check=bias_mm,
                            )
                        dst = st2["hT"][:sz, j, tg * 4 : tg * 4 + 4]
                        if bias_mm:
                            nc.tensor.matmul(
                                ps[:sz, :], onesr[:, :sz], bc4r[:],
                                start=False, stop=True, skip_group_check=True,
                            )
                            nc.scalar.activation(dst, pv[:sz], ACTF.Copy)
                        else:
                            nc.vector.tensor_tensor(
                                dst, pv[:sz], bcv[:sz], op=OP.add
                            )

                for j, (n0, sz) in enumerate(CH):
                    units.append(lambda j=j, n0=n0, sz=sz: chunk(j, n0, sz))

                def hsum_unit():
                    xsum, xsumb, hsum = st["xsum"], st["xsumb"], st["hsum"]
                    nc.gpsimd.memset(xsumb[:, N : N + 1], 0.0)
                    nc.scalar.activation(xsumb[:, :N], xsum[:], ACTF.Copy)
                    for f0, fs in MCH:
                        ps = mmps.tile([128, 512], F32, tag="mm")
                        nc.tensor.matmul(
                            ps[:, :fs], convwTb[:], xsumb[:, f0 : f0 + fs],
                            start=True, stop=True,
                        )
                        real = min(fs, N - f0)  # no bias on the virtual col
                        nc.vector.tensor_scalar(
                            hsum[:, f0 : f0 + real], ps[:, :real],
                            convb12p[:], None, op0=OP.add,
                        )
                        if real < fs:
                            nc.vector.tensor_copy(
                                hsum[:, f0 + real : f0 + fs], ps[:, real:fs]
                            )

                units.append(hsum_unit)
                return units

            # ================= stage B: adjacency + top-k ==================
            def B_units(s):
                st = states[s]

                def chunk(j, n0, sz):
                    hsum, adjB = st["hsum"], st["adjB"]
                    lhs = hsum[:, n0 : n0 + sz]
                    e1 = scrp.tile([128, N + 1], F32, tag="scr")
                    e2 = scrp.tile([128, N + 1], F32, tag="scr")
                    for (f0, fs), rt, rhs in (
                        (MCH[0], e1, memp), (MCH[1], e1, memp),
                        (MCH[0], e2, hsum), (MCH[1], e2, hsum),
                    ):
                        ps = mmps.tile([128, 512], F32, tag="mm")
                        nc.tensor.matmul(
                            ps[:sz, :fs], lhs, rhs[:, f0 : f0 + fs],
                            start=True, stop=True,
                        )
                        # relu(s*scale) -- matches reference op order
                        nc.scalar.activation(
                            rt[:sz, f0 : f0 + fs], ps[:sz, :fs], ACTF.Relu,
                            scale=SCALE,
                        )
                    # max-free softmax pieces: e = exp(relu), z from accum
                    z1 = colp.tile([128, 1], F32, tag="z1")
                    z2 = colp.tile([128, 1], F32, tag="z2")
                    z3 = colp.tile([128, 1], F32, tag="z3")
                    s0 = colp.tile([128, 1], F32, tag="s0")
                    s1 = colp.tile([128, 1], F32, tag="s1")
                    nc.scalar.activation(
                        e1[:sz], e1[:sz], ACTF.Exp, accum_out=z1[:sz]
                    )
                    nc.scalar.activation(
                        e2[:sz], e2[:sz], ACTF.Exp, accum_out=z2[:sz]
                    )
                    # z excludes the virtual column's exp(0)=1
                    nc.vector.tensor_sub(z1[:sz], z1[:sz], e1[:sz, N : N + 1])
                    nc.vector.tensor_sub(z2[:sz], z2[:sz], e2[:sz, N : N + 1])
                    # softmax1/2 normalization folds into combine scalars
                    nc.gpsimd.normalize_recip(s0[:sz], fcw0[:sz], z1[:sz])
                    nc.gpsimd.normalize_recip(s1[:sz], fcw1[:sz], z2[:sz])
                    t2 = scrp.tile([128, N + 1], F32, tag="scr")
                    nc.scalar.activation(
                        t2[:sz], e2[:sz], ACTF.Copy, scale=s1[:sz]
                    )
                    nc.vector.scalar_tensor_tensor(
                        e1[:sz], e1[:sz], s0[:sz], t2[:sz],
                        op0=OP.mult, op1=OP.add,
                    )
                    e3 = e1
                    nc.scalar.activation(
                        e3[:sz], e3[:sz], ACTF.Exp, bias=fcb[:sz],
                        accum_out=z3[:sz],
                    )
                    nc.vector.tensor_sub(z3[:sz], z3[:sz], e3[:sz, N : N + 1])
                    thr = e3[:sz, N : N + 1]
                    # ---- top-k mask on unnormalized e3 (scale-invariant) ----
                    gt = t2  # reuse
                    cnt = colp.tile([128, 1], F32, tag="cnt")
                    nc.vector.tensor_scalar(
                        gt[:sz, :N], e3[:sz, :N], thr, 0.0,
                        op0=OP.is_gt, op1=OP.add, accum_out=cnt[:sz],
                    )
                    eqh = scrhp.tile([128, N], F16, tag="eqh")
                    cumh = scrhp.tile([128, N], F16, tag="cumh")
                    nc.vector.tensor_scalar(
                        eqh[:sz], e3[:sz, :N], thr, None, op0=OP.is_equal
                    )
                    # cum = cnt + prefix(eq); keep ties while cum <= K (fp16
                    # stays exact: values are integers <= 883 < 2048)
                    nc.vector.tensor_tensor_scan(
                        cumh[:sz], eqh[:sz], zerosh[:sz],
                        initial=cnt[:sz], op0=OP.add, op1=OP.add,
                    )
                    nc.vector.scalar_tensor_tensor(
                        eqh[:sz], cumh[:sz], float(K), eqh[:sz],
                        op0=OP.is_le, op1=OP.mult,
                    )
                    nc.vector.tensor_add(gt[:sz, :N], gt[:sz, :N], eqh[:sz])
                    # adjB = mask * (ASCL/z3) * e3, written fp8 in one STT
                    s3 = colp.tile([128, 1], F32, tag="s3")
                    nc.gpsimd.normalize_recip(s3[:sz], c64[:sz], z3[:sz])
                    nc.vector.scalar_tensor_tensor(
                        adjB[:sz, j, :N], gt[:sz, :N], s3[:sz], e3[:sz, :N],
                        op0=OP.mult, op1=OP.mult,
                    )

                return [
                    (lambda j=j, n0=n0, sz=sz: chunk(j, n0, sz))
                    for j, (n0, sz) in enumerate(CH)
                ]

            # ============ stages C/D/E: diffusion + projection =============
            def CDE_units(s):
                st = states[s]
                units = []

                def c_group(kk, m0, msz, fi, f0, fs):
                    hT, g1T, adjB = st["hT"], st["g1T"], st["adjB"]
                    ps = mmps.tile([128, 512], F32, tag="mm")
                    hTf = hT[:].rearrange("p j t c -> p j (t c)")
                    for k in range(NCH // 2):  # chunk pairs, DoubleRow
                        nc.tensor.matmul(
                            ps[:msz, :fs],
                            adjB[:, 2 * k : 2 * k + 2, m0 : m0 + msz],
                            hTf[:, 2 * k : 2 * k + 2, f0 : f0 + fs],
                            start=(k == 0), stop=False,
                            perf_mode=DR, skip_group_check=True,
                        )
                    szl = CH[NCH - 1][1]  # odd last chunk, plain fp8
                    nc.tensor.matmul(
                        ps[:msz, :fs], adjB[:szl, NCH - 1, m0 : m0 + msz],
                        hTf[:szl, NCH - 1, f0 : f0 + fs],
                        start=False, stop=True, skip_group_check=True,
                    )
                    # g1 = psum/ASCL (adjB carries the x64 pre-scale)
                    dv = g1T[:msz, kk].rearrange("p t c -> p (t c)")
                    if fi == 1:
                        nc.vector.tensor_scalar(
                            dv[:, f0 : f0 + fs], ps[:msz, :fs],
                            1.0 / ASCL, None, op0=OP.mult,
                        )
                    else:
                        nc.scalar.activation(
                            dv[:, f0 : f0 + fs], ps[:msz, :fs], ACTF.Copy,
                            scale=1.0 / ASCL,
                        )

                for kk, (m0, msz) in enumerate(CH):
                    for fi, (f0, fs) in enumerate(_fch(CT)):
                        units.append(
                            lambda kk=kk, m0=m0, msz=msz, fi=fi, f0=f0, fs=fs:
                                c_group(kk, m0, msz, fi, f0, fs)
                        )

                def d_group(t, fi, f0, fs):
                    g1T, g2C, adjB = st["g1T"], st["g2C"], st["adjB"]
                    ps = mmps.tile([128, 512], F32, tag="mm")
                    for k in range(NCH // 2):  # chunk pairs, DoubleRow
                        nc.tensor.matmul(
                            ps[:, :fs], g1T[:, 2 * k : 2 * k + 2, t, :],
                            adjB[:, 2 * k : 2 * k + 2, f0 : f0 + fs],
                            start=(k == 0), stop=False,
                            perf_mode=DR, skip_group_check=True,
                        )
                    szl = CH[NCH - 1][1]
                    nc.tensor.matmul(
                        ps[:, :fs], g1T[:szl, NCH - 1, t, :],
                        adjB[:szl, NCH - 1, f0 : f0 + fs],
                        start=False, stop=True, skip_group_check=True,
                    )
                    # node-chunked g2C: contiguous 128-wide runs per chunk;
                    # g2 = psum/ASCL (adjB carries the x64 pre-scale)
                    eng = nc.scalar if (t + fi) % 2 == 0 else nc.vector
                    def ev(dst, src):
                        if eng is nc.scalar:
                            nc.scalar.activation(
                                dst, src, ACTF.Copy, scale=1.0 / ASCL
                            )
                        else:
                            nc.vector.tensor_scalar(
                                dst, src, 1.0 / ASCL, None, op0=OP.mult
                            )
                    if fi == 0:
                        ev(g2C[:, 0:4, t, :],
                           ps[:, :512].rearrange("p (j m) -> p j m", m=128))
                    else:
                        ev(g2C[:, 4:6, t, :],
                           ps[:, :256].rearrange("p (j m) -> p j m", m=128))
                        ev(g2C[:, 6, t, : N - 512 - 256], ps[:, 256 : N - 512])

                for t in range(T):
                    for fi, (f0, fs) in enumerate(NF):
                        units.append(
                            lambda t=t, fi=fi, f0=f0, fs=fs:
                                d_group(t, fi, f0, fs)
                        )

                def e_chunk(j, n0, sz):
                    g1T, g2C = st["g1T"], st["g2C"]
                    gc1 = gcmp.tile([128, 128, T], BF16, tag="gc1")
                    for th in range(2):
                        # fp8 PE transpose requires output element step 2
                        tq = tpps.tile([128, T // 2, 128, 2], FP8, tag="tp")
                        for tt in range(T // 2):
                            nc.tensor.transpose(
                                tq[:, tt, :sz, 0],
                                g1T[:sz, j, th * 6 + tt, :],
                                identf8[:sz, :sz],
                            )
                        src = tq[:, :, :sz, 0].rearrange("p t m -> p m t")
                        dst = gc1[:, :sz, th * 6 : th * 6 + 6]
                        if th == 0:
                            nc.scalar.activation(dst, src, ACTF.Copy)
                        else:
                            nc.vector.tensor_copy(dst, src)
                    g1v = gc1[:, :sz].rearrange("p n t -> p (n t)")
                    g2m = g2C[:, j].rearrange("p t m -> p m t")
                    ow = outwp.tile([128, CT], F32, tag="ow")
                    x2 = xinp.tile([128, CT], F32, tag="x2")
                    nc.sync.dma_start(
                        x2[:, : sz * T], st["xf"][:, n0 * T : (n0 + sz) * T]
                    )
                    # x2 += gcn_b*emb (per-partition), so eviction is one STT
                    nc.scalar.activation(
                        x2[:, : sz * T], x2[:, : sz * T], ACTF.Identity,
                        bias=gbe[:],
                    )
                    # node-aligned pieces (<=42 nodes = 504 psum cols)
                    for a0 in range(0, sz, 42):
                        an = min(42, sz - a0)
                        f0, fs = a0 * T, an * T
                        ps = mmps.tile([128, 512], F32, tag="mm")
                        nc.tensor.matmul(
                            ps[:, :fs], gw1T[:], g1v[:, f0 : f0 + fs],
                            start=True, stop=False, skip_group_check=True,
                        )
                        nc.tensor.matmul(
                            ps[:, :fs], gw2T[:],
                            g2m[:, a0 : a0 + an, :],
                            start=False, stop=True, skip_group_check=True,
                        )
                        # ow = z*emb + (x + gcn_b*emb)  in one pass
                        nc.vector.scalar_tensor_tensor(
                            ow[:, f0 : f0 + fs], ps[:, :fs], embx[:],
                            x2[:, f0 : f0 + fs], op0=OP.mult, op1=OP.add,
                        )
                    nc.sync.dma_start(
                        st["yf"][:, n0 * T : (n0 + sz) * T], ow[:, : sz * T]
                    )

                for j, (n0, sz) in enumerate(CH):
                    units.append(lambda j=j, n0=n0, sz=sz: e_chunk(j, n0, sz))
                return units

            # ============== software-pipelined emission ====================
            for u in A_units(0):
                u()
            for k in range(SPC + 1):
                bu = B_units(k) if k < SPC else []
                cde = CDE_units(k - 1) if k > 0 else []
                ci = 0
                step = max(1, (len(cde) + max(1, len(bu)) - 1) // max(1, len(bu)))
                for i, u in enumerate(bu):
                    u()
                    take = min(step, len(cde) - ci)
                    for _ in range(take):
                        cde[ci]()
                        ci += 1
                while ci < len(cde):
                    cde[ci]()
                    ci += 1
                if k + 1 < SPC:
                    for u in A_units(k + 1):
                        u()
                if k >= 1:
                    del states[k - 1]
    nc.compile()
    return nc


_NC = None


def _get_nc():
    global _NC
    if _NC is None:
        _NC = build_nc()
    return _NC


def make_in_maps(inputs):
    x = np.ascontiguousarray(np.asarray(inputs["x"], dtype=np.float32))
    conv_w = np.asarray(inputs["conv_w"], np.float32)
    conv_b = np.asarray(inputs["conv_b"], np.float32)
    memory = np.ascontiguousarray(np.asarray(inputs["memory"], np.float32))
    fc_w = np.asarray(inputs["fc_w"], np.float32)
    fc_b = np.asarray(inputs["fc_b"], np.float32)
    gcn_w = np.asarray(inputs["gcn_w"], np.float32)
    gcn_b = np.asarray(inputs["gcn_b"], np.float32)
    emb = np.asarray(inputs["emb"], np.float32).reshape(C)

    bcT4 = np.broadcast_to(
        conv_b[None, None, :], (128, 4, C)
    ).reshape(128, 4 * C)
    shared = {
        "convwTb": np.ascontiguousarray(conv_w.T).astype(ml_dtypes.bfloat16),
        "bcT4": np.ascontiguousarray(bcT4).astype(ml_dtypes.bfloat16),
        "convb12p": (T * conv_b).reshape(C, 1).copy(),
        "memory": memory,
        "fcw0": np.full((C, 1), fc_w[0, 0], np.float32),
        "fcw1": np.full((C, 1), fc_w[0, 1], np.float32),
        "fcb": np.full((C, 1), fc_b[0], np.float32),
        "gw1T": np.ascontiguousarray(gcn_w[:, :C].T).astype(ml_dtypes.bfloat16),
        "gw2T": np.ascontiguousarray(gcn_w[:, C:].T).astype(ml_dtypes.bfloat16),
        "embx": emb.reshape(C, 1).copy(),
        "gbe": (gcn_b * emb).reshape(C, 1).copy(),
    }
    return [
        {"x": np.ascontiguousarray(x[c * SPC : (c + 1) * SPC]), **shared}
        for c in range(NCORES)
    ]


def kernel(**inputs) -> np.ndarray:
    nc = _get_nc()
    in_maps = make_in_maps(inputs)
    res = run_bass_kernel_spmd(nc, in_maps, list(range(NCORES)))
    outs = [res.results[c]["y"] for c in range(NCORES)]
    return np.concatenate(outs, axis=0).astype(np.float32)
